# revision 1
# baseline (speedup 1.0000x reference)
"""AFNO2D block-diagonal spectral MLP kernel for 8 Trainium2 NeuronCores.

Math (after simplification of the reference):
  H = W = 128, nb = 8, bs = 96; kept == W so mode truncation is a no-op and
  the imaginary output o2i is discarded by the reference.
  With A1 = w1[0]+w1[1], D1 = w1[0]-w1[1] (same for layer 2):
    o1r = relu(Xk @ (A1/2) + Xn @ (D1/2) + b1[0]/2)
    o1i = relu(Xk @ (D1/2) - Xn @ (D1/2) + b1[1]/2)
    z   = o1r @ (A2/2) + o1i @ (D2/2) + b2[0]/2
    out = x + softshrink(z, 0.01)
  where Xn[b,i,j] = x[b, -i mod H, -j mod W] (pure permutation, done on host
  during sharding). softshrink(z) = relu(z-l) - relu(-z-l)
                                  = relu(z-l) + min(z+l, 0).

Sharding: data-parallel over the 65536 (b,i,j) sites, 8192 per core.

Mirror pairing: sites s and mirror(s) swap (Xk, Xn), so a tile T and its
elementwise-mirror tile T~ share both input tiles, and
Q = Xk@D1h - Xn@D1h satisfies Q(T~) = -Q(T): the o1i matmuls are computed
once per pair. Per 512-site tile that gives 5 matmuls instead of 6 and
halves input DMA. Mirror-fixed sites (i,j in {0,64}) and leftovers go to
two unpaired tiles per core that ship Xn explicitly.

All 0.5 scales fold into the bf16 weights; biases are per-partition bias
APs on the PSUM readouts (o1r readouts for a pair share one bias, so one
merged [96,1024] op serves both tiles).
"""

import numpy as np
import ml_dtypes

import concourse.bass as bass
import concourse.mybir as mybir
from concourse import bacc
from concourse.tile import TileContext
from concourse import bass_utils

BF16 = mybir.dt.bfloat16
F32 = mybir.dt.float32
AF = mybir.ActivationFunctionType
ALU = mybir.AluOpType

B, N, C = 4, 16384, 768
H = W = 128
NB, BS = 8, 96
LAMBDA = 0.01
NCORES = 8
SITES = B * N                      # 65536
SPC = SITES // NCORES              # 8192 sites per core
TILE = 512
FD = 2 * TILE                      # free dim of one group (a tile pair)
NGRP = SPC // FD                   # 8 groups per core
NPAIRS = 7                         # groups 0..6 are mirror pairs
UNP = FD                           # group 7: unpaired tail

_cache = {}


def _build():
    nc = bacc.Bacc("TRN2", target_bir_lowering=False)

    # per-group 2D-contiguous layouts: [group, channel, NB*1024]
    xk_d = nc.dram_tensor("xk", [NGRP, BS, NB * FD], BF16, kind="ExternalInput")
    xn_d = nc.dram_tensor("xn", [BS, NB * FD], BF16, kind="ExternalInput")
    # weight kinds (K=96): A1h, D1h, nD1h, A2h, D2h, nD2h
    w_d = nc.dram_tensor("w", [BS, NB * 6 * BS], BF16, kind="ExternalInput")
    # bias kinds: b1r, b1i, bias_a (b2/2-l), bias_m (b2/2+l), bias_bm -(b2/2+l)
    bias_d = nc.dram_tensor("b", [BS, NB * 5], F32, kind="ExternalInput")
    out_d = nc.dram_tensor("out", [NGRP, BS, NB * FD], BF16, kind="ExternalOutput")

    with TileContext(nc) as tc:
        with (
            tc.tile_pool(name="consts", bufs=1) as consts,
            tc.tile_pool(name="io", bufs=4) as io_pool,
            tc.tile_pool(name="acts", bufs=5) as act_pool,
            tc.tile_pool(name="acts2", bufs=5) as act2_pool,
            tc.tile_pool(name="psum", bufs=3, space="PSUM") as psum_pool,
            tc.tile_pool(name="psq", bufs=2, space="PSUM") as psq_pool,
        ):
            wsb = consts.tile([BS, NB * 6 * BS], BF16)
            # blocks 0-1 weights first so the first matmuls aren't gated on
            # the full weight transfer
            nc.gpsimd.dma_start(wsb[:, 0:2 * 6 * BS], w_d[:, 0:2 * 6 * BS])
            nc.gpsimd.dma_start(wsb[:, 2 * 6 * BS:], w_d[:, 2 * 6 * BS:])
            bsb = consts.tile([BS, NB * 5], F32)
            nc.gpsimd.dma_start(bsb[:], bias_d[:])

            def wAP(n, kind):
                return wsb[:, (n * 6 + kind) * BS:(n * 6 + kind + 1) * BS]

            def bAP(n, kind):
                return bsb[:, n * 5 + kind:n * 5 + kind + 1]

            A1h, D1h, nD1h, A2h, D2h, nD2h = range(6)
            Br, Bi, Ba, Bm, Bbm = range(5)

            DVE_M = (0, 2, 4, 6)   # blocks whose m readout runs on VectorE

            def l2_readouts(n, p2):
                a_t = act2_pool.tile([BS, FD], BF16, tag="a")
                nc.scalar.activation(a_t, p2, AF.Relu, bias=bAP(n, Ba), scale=1.0)
                m_t = act2_pool.tile([BS, FD], BF16, tag="m")
                if n in DVE_M:
                    nc.vector.tensor_scalar(m_t, p2, bAP(n, Bm), 0.0, ALU.add, ALU.min)
                else:
                    nc.scalar.activation(m_t, p2, AF.Relu, bias=bAP(n, Bbm), scale=-1.0)
                return a_t, m_t

            def l2_combine(j, n, a_t, m_t):
                uv, out_t, _ = group_tiles[j]
                ss_t = act2_pool.tile([BS, FD], BF16, tag="ss")
                nc.vector.tensor_tensor(ss_t, a_t, m_t,
                                        ALU.add if n in DVE_M else ALU.subtract)
                nc.gpsimd.tensor_tensor(out_t[:, n, :], ss_t, uv[:, n, :], ALU.add)
                lo, hi = n * FD, (n + 1) * FD
                nc.sync.dma_start(out_d[j, :, lo:hi],
                                  out_t.rearrange("c n s -> c (n s)")[:, lo:hi])

            # ---- software-pipelined emission over all (group, block) units ----
            # Emission order (per engine queues are in-order): L1 of unit k+1
            # is emitted before L2 of unit k so a blocked L2 matmul never
            # stalls the next unit's independent L1 matmuls.
            group_tiles = {}

            def load_group(j, split=1):
                uv = io_pool.tile([BS, NB, FD], BF16, tag="uv")
                out_t = io_pool.tile([BS, NB, FD], BF16, tag="out")
                uvf = uv.rearrange("c n s -> c (n s)")
                step = NB * FD // split
                for k in range(split):
                    nc.sync.dma_start(uvf[:, k * step:(k + 1) * step],
                                      xk_d[j, :, k * step:(k + 1) * step])
                xn_t = None
                if j == NPAIRS:
                    xn_t = consts.tile([BS, NB, FD], BF16)
                    nc.sync.dma_start(xn_t.rearrange("c n s -> c (n s)"), xn_d[:])
                group_tiles[j] = (uv, out_t, xn_t)

            def emit_l1(j, n):
                uv, out_t, xn_t = group_tiles[j]
                o1r = act_pool.tile([BS, FD], BF16, tag="o1r")
                o1i = act_pool.tile([BS, FD], BF16, tag="o1i")
                if j < NPAIRS:
                    u = uv[:, n, 0:TILE]
                    v = uv[:, n, TILE:FD]
                    prpr = psum_pool.tile([BS, FD], F32, tag="big")
                    nc.tensor.matmul(prpr[:, 0:TILE], wAP(n, A1h), u,
                                     start=True, stop=False)
                    nc.tensor.matmul(prpr[:, 0:TILE], wAP(n, D1h), v,
                                     start=False, stop=True)
                    nc.tensor.matmul(prpr[:, TILE:FD], wAP(n, A1h), v,
                                     start=True, stop=False)
                    nc.tensor.matmul(prpr[:, TILE:FD], wAP(n, D1h), u,
                                     start=False, stop=True)
                    q = psq_pool.tile([BS, TILE], F32, tag="q")
                    nc.tensor.matmul(q, wAP(n, D1h), u, start=True, stop=False)
                    nc.tensor.matmul(q, wAP(n, nD1h), v, start=False, stop=True)
                    nc.scalar.activation(o1r, prpr, AF.Relu, bias=bAP(n, Br),
                                         scale=1.0)
                    # o1i(T) = relu(Q + b1i); o1i_neg(T~) = min(Q - b1i, 0)
                    nc.vector.tensor_scalar(o1i[:, 0:TILE], q, bAP(n, Bi), 0.0,
                                            ALU.add, ALU.max)
                    nc.vector.tensor_scalar(o1i[:, TILE:FD], q, bAP(n, Bi), 0.0,
                                            ALU.subtract, ALU.min)
                else:
                    prpi_r = psum_pool.tile([BS, FD], F32, tag="big")
                    q2 = psq_pool.tile([BS, TILE], F32, tag="q")
                    q3 = psq_pool.tile([BS, TILE], F32, tag="q")
                    for t, qq in zip(range(2), (q2, q3)):
                        sl = bass.ts(t, TILE)
                        xk_s = uv[:, n, sl]
                        xn_s = xn_t[:, n, sl]
                        nc.tensor.matmul(prpi_r[:, sl], wAP(n, A1h), xk_s,
                                         start=True, stop=False)
                        nc.tensor.matmul(prpi_r[:, sl], wAP(n, D1h), xn_s,
                                         start=False, stop=True)
                        nc.tensor.matmul(qq, wAP(n, D1h), xk_s,
                                         start=True, stop=False)
                        nc.tensor.matmul(qq, wAP(n, nD1h), xn_s,
                                         start=False, stop=True)
                    nc.scalar.activation(o1r, prpi_r, AF.Relu, bias=bAP(n, Br),
                                         scale=1.0)
                    for t, qq in zip(range(2), (q2, q3)):
                        sl = bass.ts(t, TILE)
                        nc.vector.tensor_scalar(o1i[:, sl], qq, bAP(n, Bi), 0.0,
                                                ALU.add, ALU.max)
                return o1r, o1i

            def emit_l2(j, n, o1r, o1i):
                uv, out_t, _ = group_tiles[j]
                d2kind = nD2h if j < NPAIRS else D2h
                p2 = psum_pool.tile([BS, FD], F32, tag="big")
                nc.tensor.matmul(p2[:, 0:TILE], wAP(n, A2h), o1r[:, 0:TILE],
                                 start=True, stop=False)
                nc.tensor.matmul(p2[:, 0:TILE], wAP(n, D2h), o1i[:, 0:TILE],
                                 start=False, stop=True)
                nc.tensor.matmul(p2[:, TILE:FD], wAP(n, A2h), o1r[:, TILE:FD],
                                 start=True, stop=False)
                nc.tensor.matmul(p2[:, TILE:FD], wAP(n, d2kind), o1i[:, TILE:FD],
                                 start=False, stop=True)
                return l2_readouts(n, p2)

            units = [(j, n) for j in range(NGRP) for n in range(NB)]
            s1 = s2 = None   # two-stage skew: L1(k) | L2-readouts(k-1) | combine(k-2)
            for k, (j, n) in enumerate(units):
                if n == 0:
                    load_group(j, split=8 if k == 0 else 1)
                cur = (j, n, *emit_l1(j, n))
                if s2 is not None:
                    l2_combine(*s2)
                if s1 is not None:
                    s2 = (s1[0], s1[1], *emit_l2(*s1))
                s1 = cur
            l2_combine(*s2)
            s2 = (s1[0], s1[1], *emit_l2(*s1))
            l2_combine(*s2)

    nc.finalize()
    return nc


def _site_order():
    """Global site ordering: per core, 7 mirror-paired tile-pairs then a
    1024-site unpaired tail."""
    b = np.arange(SITES) // N
    ij = np.arange(SITES) % N
    i, jj = ij // W, ij % W
    midx = b * N + ((-i) % H) * W + ((-jj) % W)
    s = np.arange(SITES)
    firsts = s[s < midx]                      # 32760 pair firsts
    fixed = s[s == midx]                      # 16 self-mirrored
    per_core_paired = NPAIRS * TILE           # 3584 pairs per core
    order = np.empty((NCORES, SPC), dtype=np.int64)
    rem = firsts[NCORES * per_core_paired:]   # 4088 leftover pairs
    rem_per_core = len(rem) // NCORES         # 511
    fix_per_core = len(fixed) // NCORES       # 2
    for c in range(NCORES):
        f = firsts[c * per_core_paired:(c + 1) * per_core_paired]
        m = midx[f]
        paired = np.stack([f.reshape(NPAIRS, TILE), m.reshape(NPAIRS, TILE)],
                          axis=1).reshape(-1)
        r = rem[c * rem_per_core:(c + 1) * rem_per_core]
        fx = fixed[c * fix_per_core:(c + 1) * fix_per_core]
        tail = np.concatenate([r, midx[r], fx])
        order[c] = np.concatenate([paired, tail])
    return order.reshape(-1)


def _host_prep(x, w1, b1, w2, b2):
    bf = ml_dtypes.bfloat16
    order = _cache.setdefault("order", _site_order())
    xf = x.reshape(SITES, C)

    # xk in per-group contiguous layout [core, group, 96, NB*FD]
    xperm = xf[order].T.astype(bf)                     # [C, SITES]
    xk_all = np.ascontiguousarray(
        xperm.reshape(NB, BS, NCORES, NGRP, FD).transpose(2, 3, 1, 0, 4)
    ).reshape(NCORES, NGRP, BS, NB * FD)

    b_ = order // N
    ij = order % N
    i, jj = ij // W, ij % W
    morder = (b_ * N + ((-i) % H) * W + ((-jj) % W)).reshape(NCORES, SPC)
    un_idx = morder[:, NPAIRS * FD:].reshape(-1)
    xn_all = np.ascontiguousarray(
        xf[un_idx].T.astype(bf).reshape(NB, BS, NCORES, UNP).transpose(2, 1, 0, 3)
    ).reshape(NCORES, BS, NB * UNP)

    A1h = (w1[0] + w1[1]) * 0.5               # [NB, in, out]
    D1h = (w1[0] - w1[1]) * 0.5
    A2h = (w2[0] + w2[1]) * 0.5
    D2h = (w2[0] - w2[1]) * 0.5
    wpack = np.empty((BS, NB * 6 * BS), dtype=np.float32)
    for n in range(NB):
        for k, mat in enumerate((A1h[n], D1h[n], -D1h[n], A2h[n], D2h[n], -D2h[n])):
            wpack[:, (n * 6 + k) * BS:(n * 6 + k + 1) * BS] = mat
    wpack = wpack.astype(bf)

    bpack = np.empty((BS, NB * 5), dtype=np.float32)
    for n in range(NB):
        bpack[:, n * 5 + 0] = b1[0, n] * 0.5
        bpack[:, n * 5 + 1] = b1[1, n] * 0.5
        bpack[:, n * 5 + 2] = b2[0, n] * 0.5 - LAMBDA
        bpack[:, n * 5 + 3] = b2[0, n] * 0.5 + LAMBDA
        bpack[:, n * 5 + 4] = -(b2[0, n] * 0.5 + LAMBDA)

    in_maps = []
    for c in range(NCORES):
        in_maps.append({
            "xk": np.ascontiguousarray(xk_all[c]),
            "xn": np.ascontiguousarray(xn_all[c]),
            "w": wpack,
            "b": bpack,
        })
    return in_maps


def _assemble(results):
    order = _cache["order"]
    # out per core: [NGRP, BS, NB*FD] -> [C, SPC] in site order
    cols = np.concatenate(
        [r["out"].reshape(NGRP, BS, NB, FD).transpose(2, 1, 0, 3).reshape(C, SPC)
         for r in results], axis=1)
    full = np.empty((SITES, C), dtype=np.float32)
    full[order] = cols.T.astype(np.float32)
    return full.reshape(B, N, C)


def _run(x, w1, b1, w2, b2, trace=False):
    if "nc" not in _cache:
        _cache["nc"] = _build()
    nc = _cache["nc"]
    in_maps = _host_prep(x, w1, b1, w2, b2)
    res = bass_utils.run_bass_kernel_spmd(
        nc, in_maps, core_ids=list(range(NCORES)), trace=trace)
    return _assemble(res.results), res


def kernel(x, w1, b1, w2, b2):
    out, _ = _run(x, w1, b1, w2, b2, trace=False)
    return out



# revision 9
# speedup vs baseline: 1.0878x; 1.0878x over previous
"""AFNO2D block-diagonal spectral MLP kernel for 8 Trainium2 NeuronCores.

Math (after simplification of the reference; see reference.py):
  H = W = 128, nb = 8, bs = 96; kept == W so mode truncation is a no-op and
  the imaginary output o2i is discarded.  With halves folded into weights:
    o1r = relu(Xk@(w10/2) + Xn@(w11/2)... ) etc.  For a mirror pair of site
  tiles T, T~ (Xk/Xn swap), define S = x(T)+x(T~), D = x(T)-x(T~) (host):
    o1r(T)  = relu(P + R + b1r)      P = S@(w10/2), R = D@(w11/2)
    o1r(T~) = relu(P - R + b1r)
    o1i(T)  = relu(Q + b1i)          Q = D@D1h, D1h = (w10-w11)/2
    o1i(T~) = relu(-Q + b1i)
    z(T)    = o1r@A2h + o1i@D2h + b2r     (A2h/D2h = (w2[0]±w2[1])/2)
    out     = x + softshrink(z, 0.01)     (residual added on host)

Device mapping per (group, block) unit (1024 sites = 512 mirror pairs):
  5 fp8 DoubleRow matmuls (K pairs (S[p], D[p]) + a ones-row riding biases):
    A = P+R+b1r | C = Q+b1i  -> one adjacent psum pair, drained by ONE
    ScalarE relu into the fp8 interleaved (o1r|o1i) layer-2 moving tile.
    B = P-R+b1r  (ScalarE relu), o1i~ from C (VectorE/ScalarE), then
    zT/zTn DoubleRow matmuls re-use the A|C psum banks, drained by a
    custom DVE softshrink op (x - clamp(x, -l, l)) straight to fp8.
  The 16 self-mirror sites (i,j in {0,64}) are computed on the host; one
  duplicated pair pads each core's 4095 pairs to 8 groups x 512 columns.
"""

import numpy as np
import ml_dtypes

import concourse.bass as bass
import concourse.mybir as mybir
from concourse import bacc
from concourse.tile import TileContext
from concourse import bass_utils
from concourse import dve_ops as _dve_ops
from concourse.dve_spec import Spec, Src0, C0, C1, maxx, minn, lower as _dve_lower
from concourse.dve_uop import DveOpSpec
from concourse.bass_utils import dve_ver_for

FP8 = mybir.dt.float8e4
F32 = mybir.dt.float32
AF = mybir.ActivationFunctionType
ALU = mybir.AluOpType
NPF8 = ml_dtypes.float8_e4m3   # TRN FP8_EXP4-compatible (max 240, has inf)

B, N, C = 4, 16384, 768
H = W = 128
NB, BS = 8, 96
P = BS + 1                     # 96 channels + ones row (bias riding)
LAM = 0.01
NCORES = 8
SITES = B * N                  # 65536
TILE = 512                     # mirror pairs per unit
NGRP = 8                       # groups per core; NGRP*TILE = 4096 pair cols
PAIRS = 4095                   # real pairs per core (+1 duplicated pad col)
WPAD = 112                     # weight free-dim pad so pair stride % 16 == 0
ACT_I_BLOCKS = (0, 4)          # blocks whose o1i~ readout runs on ScalarE
MODE = "reg9"                  # "dr5" DoubleRow 5-MM | "reg9" regular-fp8 9-MM

_cache = {}


def _register_softshrink():
    name = "SOFTSHRINK_ANT"
    for op in _dve_ops.OPS:
        if op.name == name:
            return op
    spec = Spec(
        body=Src0 - minn(maxx(Src0, C0), C1),
        reference=lambda in0, in1, s0, s1, imm2: (
            in0.astype(np.float32) - np.clip(in0.astype(np.float32), s0, s1)
        ),
    )
    row = max(_dve_ops._SUB_OPCODE_FOR_NAME.values()) + 1
    assert row < 0x20
    _dve_ops._SUB_OPCODE_FOR_NAME[name] = row
    ver = dve_ver_for("TRN2")
    sha = DveOpSpec(
        name=name, opcode=row, uops=_dve_lower(spec, ver=ver), rd1_en=False
    ).sha(ver)
    op = _dve_ops.DveOp(name, spec, subdim=False, uops_sha={ver: sha})
    _dve_ops.OPS.append(op)
    _dve_ops.CUSTOM_DVE_SPECS[name] = spec
    return op


SOFTSHRINK = _register_softshrink()


def _build():
    nc = bacc.Bacc("TRN2", target_bir_lowering=False)

    NW = 5 * 2 if MODE == "dr5" else 7
    xd = nc.dram_tensor("x", [NGRP, P, NB * 2 * TILE], FP8, kind="ExternalInput")
    wd = nc.dram_tensor("w", [P, NB * NW * WPAD], FP8, kind="ExternalInput")
    bd = nc.dram_tensor("b", [P, NB], F32, kind="ExternalInput")
    outd = nc.dram_tensor("out", [NGRP, BS, NB * 2 * TILE], FP8,
                          kind="ExternalOutput")

    DR = mybir.MatmulPerfMode.DoubleRow
    A_, B_, C_, ZT_, ZN_ = range(5)

    with TileContext(nc) as tc:
        with (
            tc.tile_pool(name="consts", bufs=1) as consts,
            tc.tile_pool(name="io", bufs=3) as io_pool,
            tc.tile_pool(name="outp", bufs=2) as out_pool,
            tc.tile_pool(name="o1", bufs=3) as o1_pool,
            tc.tile_pool(name="psac", bufs=3, space="PSUM") as psac_pool,
            tc.tile_pool(name="psb", bufs=2, space="PSUM") as psb_pool,
        ):
            wsb = consts.tile([P, NB * NW, WPAD], FP8)
            wflat = wsb.rearrange("p a b -> p (a b)")
            sl0 = 2 * NW * WPAD        # first two blocks' weights first
            nc.gpsimd.dma_start(wflat[:, 0:sl0], wd[:, 0:sl0])
            nc.gpsimd.dma_start(wflat[:, sl0:], wd[:, sl0:])
            bsb = consts.tile([P, NB], F32)
            nc.gpsimd.dma_start(bsb[:], bd[:])

            def wAP(n, s, m):
                base = (n * 5 + s) * 2
                return wsb[:, base:base + 2, 0:m]

            def wR(n, s, m):
                return wsb[:, n * 7 + s, 0:m]

            def bAP(n):
                return bsb[:, n:n + 1]

            group_tiles = {}

            def load_group(j, split):
                sd = io_pool.tile([P, NB * 2, TILE], FP8, tag="sd")
                out_t = out_pool.tile([BS, NB, 2 * TILE], FP8, tag="out")
                sdf = sd.rearrange("p a b -> p (a b)")
                step = NB * 2 * TILE // split
                for k in range(split):
                    nc.gpsimd.dma_start(sdf[:, k * step:(k + 1) * step],
                                        xd[j, :, k * step:(k + 1) * step])
                group_tiles[j] = (sd, out_t)

            def stage1(j, n):
                sd, out_t = group_tiles[j]
                ps = psac_pool.tile([P, 2 * TILE], F32, tag="ac")
                pb = psb_pool.tile([P, TILE], F32, tag="b")
                if MODE == "dr5":
                    mv = sd[:, 2 * n:2 * n + 2, :]
                    nc.tensor.matmul(ps[:, 0:TILE], wAP(n, A_, P), mv,
                                     start=True, stop=True, perf_mode=DR)
                    nc.tensor.matmul(pb, wAP(n, B_, P), mv,
                                     start=True, stop=True, perf_mode=DR)
                    nc.tensor.matmul(ps[:, TILE:2 * TILE], wAP(n, C_, P), mv,
                                     start=True, stop=True, perf_mode=DR)
                else:
                    S = sd[:, 2 * n, :]
                    D = sd[:, 2 * n + 1, :]
                    nc.tensor.matmul(ps[:, 0:TILE], wR(n, 0, P), S,
                                     start=True, stop=False)
                    nc.tensor.matmul(ps[:, 0:TILE], wR(n, 1, P), D,
                                     start=False, stop=True)
                    nc.tensor.matmul(pb, wR(n, 0, P), S, start=True, stop=False)
                    nc.tensor.matmul(pb, wR(n, 2, P), D, start=False, stop=True)
                    nc.tensor.matmul(ps[:, TILE:2 * TILE], wR(n, 3, P), D,
                                     start=True, stop=True)
                o1T = o1_pool.tile([P, 2, TILE], FP8, tag="t")
                o1N = o1_pool.tile([P, 2, TILE], FP8, tag="tn")
                # o1r(T) | o1i(T) in one drain (biases already in psum)
                nc.scalar.activation(o1T.rearrange("p a b -> p (a b)"), ps,
                                     AF.Relu)
                nc.scalar.activation(o1N[:, 0, :], pb, AF.Relu)
                if n in ACT_I_BLOCKS:
                    # +o1i(T~) = relu(-C + 2*b1i); pairs with +D2h weights
                    nc.scalar.activation(o1N[:, 1, :], ps[:, TILE:2 * TILE],
                                         AF.Relu, bias=bAP(n), scale=-1.0)
                else:
                    # -o1i(T~) = min(C - 2*b1i, 0); pairs with -D2h weights
                    nc.vector.tensor_scalar(o1N[:, 1, :], ps[:, TILE:2 * TILE],
                                            bAP(n), 0.0, ALU.subtract, ALU.min)
                return j, n, ps, o1T, o1N

            def stage2(j, n, ps, o1T, o1N):
                _, out_t = group_tiles[j]
                # z psum re-uses the A|C banks (WAR on the stage-1 drains)
                if MODE == "dr5":
                    nc.tensor.matmul(ps[0:BS, 0:TILE], wAP(n, ZT_, BS), o1T,
                                     start=True, stop=True, perf_mode=DR)
                    nc.tensor.matmul(ps[0:BS, TILE:2 * TILE], wAP(n, ZN_, BS),
                                     o1N, start=True, stop=True, perf_mode=DR)
                else:
                    nc.tensor.matmul(ps[0:BS, 0:TILE], wR(n, 4, BS),
                                     o1T[:, 0, :], start=True, stop=False)
                    nc.tensor.matmul(ps[0:BS, 0:TILE], wR(n, 5, BS),
                                     o1T[:, 1, :], start=False, stop=True)
                    nc.tensor.matmul(ps[0:BS, TILE:2 * TILE], wR(n, 4, BS),
                                     o1N[:, 0, :], start=True, stop=False)
                    nc.tensor.matmul(ps[0:BS, TILE:2 * TILE], wR(n, 6, BS),
                                     o1N[:, 1, :], start=False, stop=True)
                nc.vector._custom_dve(SOFTSHRINK, out=out_t[:, n, :],
                                      in0=ps[0:BS, :], s0=-LAM, s1=LAM)
                lo = n * 2 * TILE
                nc.sync.dma_start(outd[j, :, lo:lo + 2 * TILE], out_t[:, n, :])

            units = [(j, n) for j in range(NGRP) for n in range(NB)]
            pend = None
            for k, (j, n) in enumerate(units):
                if n == 0:
                    load_group(j, split=8 if k == 0 else 2)
                cur = stage1(j, n)
                if pend is not None:
                    stage2(*pend)
                pend = cur
            stage2(*pend)

    nc.finalize()
    return nc


def _site_order():
    idx = np.arange(SITES)
    b = idx // N
    r = idx % N
    i, jj = r // W, r % W
    mi = b * N + ((-i) % H) * W + ((-jj) % W)
    firsts = idx[idx < mi]                    # 32760 = 8 * 4095
    fixed = idx[idx == mi]                    # 16 self-mirror sites
    F = np.empty((NCORES, NGRP * TILE), dtype=np.int64)
    for c in range(NCORES):
        fc = firsts[c * PAIRS:(c + 1) * PAIRS]
        F[c, :PAIRS] = fc
        F[c, PAIRS:] = fc[0]                  # pad col: duplicated pair
    M = mi[F]
    return F, M, fixed


def _host_prep(x, w1, b1, w2, b2):
    if "order" not in _cache:
        _cache["order"] = _site_order()
    F, M, fixed = _cache["order"]

    xf = np.ascontiguousarray(x.reshape(SITES, C))
    u = xf[F.reshape(-1)].reshape(NCORES, NGRP, TILE, NB, BS)
    v = xf[M.reshape(-1)].reshape(NCORES, NGRP, TILE, NB, BS)
    S = u + v
    Dd = u - v
    sd = np.empty((NCORES, NGRP, P, NB, 2, TILE), dtype=NPF8)
    sd[:, :, :BS, :, 0, :] = S.transpose(0, 1, 4, 3, 2).astype(NPF8)
    sd[:, :, :BS, :, 1, :] = Dd.transpose(0, 1, 4, 3, 2).astype(NPF8)
    sd[:, :, BS, :, :, :] = np.float32(1.0)

    w10h = w1[0] * 0.5
    w11h = w1[1] * 0.5
    D1h = (w1[0] - w1[1]) * 0.5
    A2h = (w2[0] + w2[1]) * 0.5
    D2h = (w2[0] - w2[1]) * 0.5
    wT = lambda m: m.transpose(1, 0, 2)       # [NB,in,out] -> [in,NB,out]
    sgn = np.where(np.isin(np.arange(NB), ACT_I_BLOCKS), 1.0, -1.0)
    if MODE == "dr5":
        wpack = np.zeros((P, NB, 5, 2, WPAD), dtype=np.float32)
        wpack[:BS, :, A_, 0, :BS] = wT(w10h)
        wpack[BS, :, A_, 0, :BS] = b1[0] * 0.5
        wpack[BS, :, A_, 0, BS] = 1.0
        wpack[:BS, :, A_, 1, :BS] = wT(w11h)
        wpack[:, :, B_, 0, :] = wpack[:, :, A_, 0, :]
        wpack[:BS, :, B_, 1, :BS] = wT(-w11h)
        wpack[BS, :, C_, 0, :BS] = b1[1] * 0.5
        wpack[BS, :, C_, 0, BS] = 1.0
        wpack[:BS, :, C_, 1, :BS] = wT(D1h)
        wpack[:BS, :, ZT_, 0, :BS] = wT(A2h)
        wpack[BS, :, ZT_, 0, :BS] = b2[0] * 0.5
        wpack[:BS, :, ZT_, 1, :BS] = wT(D2h)
        wpack[:, :, ZN_, 0, :] = wpack[:, :, ZT_, 0, :]
        wpack[:BS, :, ZN_, 1, :BS] = wT(D2h * sgn[:, None, None])
        wpack8 = wpack.reshape(P, NB * 5 * 2 * WPAD).astype(NPF8)
    else:
        wpack = np.zeros((P, NB, 7, WPAD), dtype=np.float32)
        wpack[:BS, :, 0, :BS] = wT(w10h)      # A0: + b1r row + const col
        wpack[BS, :, 0, :BS] = b1[0] * 0.5
        wpack[BS, :, 0, BS] = 1.0
        wpack[:BS, :, 1, :BS] = wT(w11h)      # A1
        wpack[:BS, :, 2, :BS] = wT(-w11h)     # B1
        wpack[:BS, :, 3, :BS] = wT(D1h)       # C1: + b1i row + const col
        wpack[BS, :, 3, :BS] = b1[1] * 0.5
        wpack[BS, :, 3, BS] = 1.0
        wpack[:BS, :, 4, :BS] = wT(A2h)       # Z0: + b2r row
        wpack[BS, :, 4, :BS] = b2[0] * 0.5
        wpack[:BS, :, 5, :BS] = wT(D2h)       # Z1
        wpack[:BS, :, 6, :BS] = wT(D2h * sgn[:, None, None])  # Z1n
        wpack8 = wpack.reshape(P, NB * 7 * WPAD).astype(NPF8)

    bpack = np.empty((P, NB), dtype=np.float32)
    bpack[:BS] = b1[1].T                      # 2*b1i
    bpack[BS] = 0.5
    in_maps = []
    for c in range(NCORES):
        in_maps.append({
            "x": np.ascontiguousarray(sd[c].reshape(NGRP, P, NB * 2 * TILE)),
            "w": wpack8,
            "b": bpack,
        })
    return in_maps


A_, B_, C_, ZT_, ZN_ = range(5)


def _fixed_out(xf, w1, b1, w2, b2, fixed):
    xs = xf[fixed].reshape(len(fixed), NB, BS)
    o1r = np.maximum(np.einsum("knp,npq->knq", xs, w1[0]) + 0.5 * b1[0], 0.0)
    o1i = np.maximum(0.5 * b1[1], 0.0)[None]
    A2h = (w2[0] + w2[1]) * 0.5
    D2h = (w2[0] - w2[1]) * 0.5
    z = (np.einsum("knp,npq->knq", o1r, A2h)
         + np.einsum("knp,npq->knq", np.broadcast_to(o1i, o1r.shape), D2h)
         + 0.5 * b2[0])
    return (z - np.clip(z, -LAM, LAM)).reshape(len(fixed), C)


def _assemble(results, x, w1, b1, w2, b2):
    F, M, fixed = _cache["order"]
    full = np.zeros((SITES, C), dtype=np.float32)
    for c in range(NCORES):
        o = results[c]["out"].reshape(NGRP, BS, NB, 2, TILE)
        t = o[:, :, :, 0, :].transpose(0, 3, 2, 1).reshape(NGRP * TILE, C)
        tn = o[:, :, :, 1, :].transpose(0, 3, 2, 1).reshape(NGRP * TILE, C)
        full[F[c]] = t.astype(np.float32)
        full[M[c]] = tn.astype(np.float32)
    xf = x.reshape(SITES, C)
    full[fixed] = _fixed_out(xf, w1, b1, w2, b2, fixed)
    return (xf + full).reshape(B, N, C)


def _run(x, w1, b1, w2, b2, trace=False):
    if "nc" not in _cache:
        _cache["nc"] = _build()
    nc = _cache["nc"]
    in_maps = _host_prep(x, w1, b1, w2, b2)
    res = bass_utils.run_bass_kernel_spmd(
        nc, in_maps, core_ids=list(range(NCORES)), trace=trace)
    return _assemble(res.results, x, w1, b1, w2, b2), res


def kernel(x, w1, b1, w2, b2):
    out, _ = _run(x, w1, b1, w2, b2, trace=False)
    return out


# revision 10
# speedup vs baseline: 1.1140x; 1.0241x over previous
"""AFNO2D block-diagonal spectral MLP kernel for 8 Trainium2 NeuronCores.

Math (after simplification of the reference; see reference.py):
  H = W = 128, nb = 8, bs = 96; kept == W so mode truncation is a no-op and
  the imaginary output o2i is discarded.  With halves folded into weights:
    o1r = relu(Xk@(w10/2) + Xn@(w11/2)... ) etc.  For a mirror pair of site
  tiles T, T~ (Xk/Xn swap), define S = x(T)+x(T~), D = x(T)-x(T~) (host):
    o1r(T)  = relu(P + R + b1r)      P = S@(w10/2), R = D@(w11/2)
    o1r(T~) = relu(P - R + b1r)
    o1i(T)  = relu(Q + b1i)          Q = D@D1h, D1h = (w10-w11)/2
    o1i(T~) = relu(-Q + b1i)
    z(T)    = o1r@A2h + o1i@D2h + b2r     (A2h/D2h = (w2[0]±w2[1])/2)
    out     = x + softshrink(z, 0.01)     (residual added on host)

Device mapping per (group, block) unit (1024 sites = 512 mirror pairs):
  5 fp8 DoubleRow matmuls (K pairs (S[p], D[p]) + a ones-row riding biases):
    A = P+R+b1r | C = Q+b1i  -> one adjacent psum pair, drained by ONE
    ScalarE relu into the fp8 interleaved (o1r|o1i) layer-2 moving tile.
    B = P-R+b1r  (ScalarE relu), o1i~ from C (VectorE/ScalarE), then
    zT/zTn DoubleRow matmuls re-use the A|C psum banks, drained by a
    custom DVE softshrink op (x - clamp(x, -l, l)) straight to fp8.
  The 16 self-mirror sites (i,j in {0,64}) are computed on the host; one
  duplicated pair pads each core's 4095 pairs to 8 groups x 512 columns.
"""

import numpy as np
import ml_dtypes

import concourse.bass as bass
import concourse.mybir as mybir
from concourse import bacc
from concourse.tile import TileContext
from concourse import bass_utils
from concourse import dve_ops as _dve_ops
from concourse.dve_spec import Spec, Src0, C0, C1, maxx, minn, lower as _dve_lower
from concourse.dve_uop import DveOpSpec
from concourse.bass_utils import dve_ver_for

FP8 = mybir.dt.float8e4
F32 = mybir.dt.float32
AF = mybir.ActivationFunctionType
ALU = mybir.AluOpType
NPF8 = ml_dtypes.float8_e4m3   # TRN FP8_EXP4-compatible (max 240, has inf)

B, N, C = 4, 16384, 768
H = W = 128
NB, BS = 8, 96
P = BS + 1                     # 96 channels + ones row (bias riding)
LAM = 0.01
NCORES = 8
SITES = B * N                  # 65536
TILE = 512                     # mirror pairs per unit
NGRP = 8                       # groups per core; NGRP*TILE = 4096 pair cols
PAIRS = 4095                   # real pairs per core (+1 duplicated pad col)
WPAD = 112                     # weight free-dim pad so pair stride % 16 == 0
ACT_I_BLOCKS = (0, 4)          # blocks whose o1i~ readout runs on ScalarE
MODE = "reg9"                  # "dr5" DoubleRow 5-MM | "reg9" regular-fp8 9-MM

_cache = {}


def _register_softshrink():
    name = "SOFTSHRINK_ANT"
    for op in _dve_ops.OPS:
        if op.name == name:
            return op
    spec = Spec(
        body=Src0 - minn(maxx(Src0, C0), C1),
        reference=lambda in0, in1, s0, s1, imm2: (
            in0.astype(np.float32) - np.clip(in0.astype(np.float32), s0, s1)
        ),
    )
    row = max(_dve_ops._SUB_OPCODE_FOR_NAME.values()) + 1
    assert row < 0x20
    _dve_ops._SUB_OPCODE_FOR_NAME[name] = row
    ver = dve_ver_for("TRN2")
    sha = DveOpSpec(
        name=name, opcode=row, uops=_dve_lower(spec, ver=ver), rd1_en=False
    ).sha(ver)
    op = _dve_ops.DveOp(name, spec, subdim=False, uops_sha={ver: sha})
    _dve_ops.OPS.append(op)
    _dve_ops.CUSTOM_DVE_SPECS[name] = spec
    return op


SOFTSHRINK = _register_softshrink()


def _build():
    nc = bacc.Bacc("TRN2", target_bir_lowering=False)

    NW = 5 * 2 if MODE == "dr5" else 7
    xd = nc.dram_tensor("x", [NGRP, P, NB * 2 * TILE], FP8, kind="ExternalInput")
    wd = nc.dram_tensor("w", [P, NB * NW * WPAD], FP8, kind="ExternalInput")
    bd = nc.dram_tensor("b", [P, NB], F32, kind="ExternalInput")
    outd = nc.dram_tensor("out", [NGRP, BS, NB * 2 * TILE], FP8,
                          kind="ExternalOutput")

    DR = mybir.MatmulPerfMode.DoubleRow
    A_, B_, C_, ZT_, ZN_ = range(5)

    with TileContext(nc) as tc:
        with (
            tc.tile_pool(name="consts", bufs=1) as consts,
            tc.tile_pool(name="io", bufs=3) as io_pool,
            tc.tile_pool(name="outp", bufs=2) as out_pool,
            tc.tile_pool(name="o1", bufs=3) as o1_pool,
            tc.tile_pool(name="psac", bufs=3, space="PSUM") as psac_pool,
            tc.tile_pool(name="psb", bufs=2, space="PSUM") as psb_pool,
        ):
            wsb = consts.tile([P, NB * NW, WPAD], FP8)
            wflat = wsb.rearrange("p a b -> p (a b)")
            sl0 = 2 * NW * WPAD        # first two blocks' weights first
            nc.gpsimd.dma_start(wflat[:, 0:sl0], wd[:, 0:sl0])
            nc.gpsimd.dma_start(wflat[:, sl0:], wd[:, sl0:])
            bsb = consts.tile([P, NB], F32)
            nc.gpsimd.dma_start(bsb[:], bd[:])

            def wAP(n, s, m):
                base = (n * 5 + s) * 2
                return wsb[:, base:base + 2, 0:m]

            def wR(n, s, m):
                return wsb[:, n * 7 + s, 0:m]

            def bAP(n):
                return bsb[:, n:n + 1]

            group_tiles = {}

            def load_group(j, split):
                sd = io_pool.tile([P, NB * 2, TILE], FP8, tag="sd")
                out_t = out_pool.tile([BS, NB, 2 * TILE], FP8, tag="out")
                sdf = sd.rearrange("p a b -> p (a b)")
                step = NB * 2 * TILE // split
                for k in range(split):
                    nc.gpsimd.dma_start(sdf[:, k * step:(k + 1) * step],
                                        xd[j, :, k * step:(k + 1) * step])
                group_tiles[j] = (sd, out_t)

            def stage1(j, n):
                sd, out_t = group_tiles[j]
                ps = psac_pool.tile([P, 2 * TILE], F32, tag="ac")
                pb = psb_pool.tile([P, TILE], F32, tag="b")
                if MODE == "dr5":
                    mv = sd[:, 2 * n:2 * n + 2, :]
                    nc.tensor.matmul(ps[:, 0:TILE], wAP(n, A_, P), mv,
                                     start=True, stop=True, perf_mode=DR)
                    nc.tensor.matmul(pb, wAP(n, B_, P), mv,
                                     start=True, stop=True, perf_mode=DR)
                    nc.tensor.matmul(ps[:, TILE:2 * TILE], wAP(n, C_, P), mv,
                                     start=True, stop=True, perf_mode=DR)
                else:
                    S = sd[:, 2 * n, :]
                    D = sd[:, 2 * n + 1, :]
                    nc.tensor.matmul(ps[:, 0:TILE], wR(n, 0, P), S,
                                     start=True, stop=False)
                    nc.tensor.matmul(ps[:, 0:TILE], wR(n, 1, P), D,
                                     start=False, stop=True)
                    nc.tensor.matmul(pb, wR(n, 0, P), S, start=True, stop=False)
                    nc.tensor.matmul(pb, wR(n, 2, P), D, start=False, stop=True)
                    nc.tensor.matmul(ps[:, TILE:2 * TILE], wR(n, 3, P), D,
                                     start=True, stop=True)
                o1T = o1_pool.tile([P, 2, TILE], FP8, tag="t")
                o1N = o1_pool.tile([P, 2, TILE], FP8, tag="tn")
                # o1r(T) | o1i(T) in one drain (biases already in psum)
                nc.scalar.activation(o1T.rearrange("p a b -> p (a b)"), ps,
                                     AF.Relu)
                nc.scalar.activation(o1N[:, 0, :], pb, AF.Relu)
                if n in ACT_I_BLOCKS:
                    # +o1i(T~) = relu(-C + 2*b1i); pairs with +D2h weights
                    nc.scalar.activation(o1N[:, 1, :], ps[:, TILE:2 * TILE],
                                         AF.Relu, bias=bAP(n), scale=-1.0)
                else:
                    # -o1i(T~) = min(C - 2*b1i, 0); pairs with -D2h weights
                    nc.vector.tensor_scalar(o1N[:, 1, :], ps[:, TILE:2 * TILE],
                                            bAP(n), 0.0, ALU.subtract, ALU.min)
                return j, n, ps, o1T, o1N

            def stage2(j, n, ps, o1T, o1N):
                _, out_t = group_tiles[j]
                # z psum re-uses the A|C banks (WAR on the stage-1 drains)
                if MODE == "dr5":
                    nc.tensor.matmul(ps[0:BS, 0:TILE], wAP(n, ZT_, BS), o1T,
                                     start=True, stop=True, perf_mode=DR)
                    nc.tensor.matmul(ps[0:BS, TILE:2 * TILE], wAP(n, ZN_, BS),
                                     o1N, start=True, stop=True, perf_mode=DR)
                else:
                    nc.tensor.matmul(ps[0:BS, 0:TILE], wR(n, 4, BS),
                                     o1T[:, 0, :], start=True, stop=False)
                    nc.tensor.matmul(ps[0:BS, 0:TILE], wR(n, 5, BS),
                                     o1T[:, 1, :], start=False, stop=True)
                    nc.tensor.matmul(ps[0:BS, TILE:2 * TILE], wR(n, 4, BS),
                                     o1N[:, 0, :], start=True, stop=False)
                    nc.tensor.matmul(ps[0:BS, TILE:2 * TILE], wR(n, 6, BS),
                                     o1N[:, 1, :], start=False, stop=True)
                nc.vector._custom_dve(SOFTSHRINK, out=out_t[:, n, :],
                                      in0=ps[0:BS, :], s0=-LAM, s1=LAM)
                lo = n * 2 * TILE
                nc.sync.dma_start(outd[j, :, lo:lo + 2 * TILE], out_t[:, n, :])

            units = [(j, n) for j in range(NGRP) for n in range(NB)]
            pend = []          # 2-unit skew: L2 of unit k issues after L1(k+2)
            for k, (j, n) in enumerate(units):
                if n == 0:
                    load_group(j, split=8 if k == 0 else 2)
                pend.append(stage1(j, n))
                if len(pend) > 2:
                    stage2(*pend.pop(0))
            for s in pend:
                stage2(*s)

    nc.finalize()
    return nc


def _site_order():
    idx = np.arange(SITES)
    b = idx // N
    r = idx % N
    i, jj = r // W, r % W
    mi = b * N + ((-i) % H) * W + ((-jj) % W)
    firsts = idx[idx < mi]                    # 32760 = 8 * 4095
    fixed = idx[idx == mi]                    # 16 self-mirror sites
    F = np.empty((NCORES, NGRP * TILE), dtype=np.int64)
    for c in range(NCORES):
        fc = firsts[c * PAIRS:(c + 1) * PAIRS]
        F[c, :PAIRS] = fc
        F[c, PAIRS:] = fc[0]                  # pad col: duplicated pair
    M = mi[F]
    return F, M, fixed


def _host_prep(x, w1, b1, w2, b2):
    if "order" not in _cache:
        _cache["order"] = _site_order()
    F, M, fixed = _cache["order"]

    xf = np.ascontiguousarray(x.reshape(SITES, C))
    u = xf[F.reshape(-1)].reshape(NCORES, NGRP, TILE, NB, BS)
    v = xf[M.reshape(-1)].reshape(NCORES, NGRP, TILE, NB, BS)
    S = u + v
    Dd = u - v
    sd = np.empty((NCORES, NGRP, P, NB, 2, TILE), dtype=NPF8)
    sd[:, :, :BS, :, 0, :] = S.transpose(0, 1, 4, 3, 2).astype(NPF8)
    sd[:, :, :BS, :, 1, :] = Dd.transpose(0, 1, 4, 3, 2).astype(NPF8)
    sd[:, :, BS, :, :, :] = np.float32(1.0)

    w10h = w1[0] * 0.5
    w11h = w1[1] * 0.5
    D1h = (w1[0] - w1[1]) * 0.5
    A2h = (w2[0] + w2[1]) * 0.5
    D2h = (w2[0] - w2[1]) * 0.5
    wT = lambda m: m.transpose(1, 0, 2)       # [NB,in,out] -> [in,NB,out]
    sgn = np.where(np.isin(np.arange(NB), ACT_I_BLOCKS), 1.0, -1.0)
    if MODE == "dr5":
        wpack = np.zeros((P, NB, 5, 2, WPAD), dtype=np.float32)
        wpack[:BS, :, A_, 0, :BS] = wT(w10h)
        wpack[BS, :, A_, 0, :BS] = b1[0] * 0.5
        wpack[BS, :, A_, 0, BS] = 1.0
        wpack[:BS, :, A_, 1, :BS] = wT(w11h)
        wpack[:, :, B_, 0, :] = wpack[:, :, A_, 0, :]
        wpack[:BS, :, B_, 1, :BS] = wT(-w11h)
        wpack[BS, :, C_, 0, :BS] = b1[1] * 0.5
        wpack[BS, :, C_, 0, BS] = 1.0
        wpack[:BS, :, C_, 1, :BS] = wT(D1h)
        wpack[:BS, :, ZT_, 0, :BS] = wT(A2h)
        wpack[BS, :, ZT_, 0, :BS] = b2[0] * 0.5
        wpack[:BS, :, ZT_, 1, :BS] = wT(D2h)
        wpack[:, :, ZN_, 0, :] = wpack[:, :, ZT_, 0, :]
        wpack[:BS, :, ZN_, 1, :BS] = wT(D2h * sgn[:, None, None])
        wpack8 = wpack.reshape(P, NB * 5 * 2 * WPAD).astype(NPF8)
    else:
        wpack = np.zeros((P, NB, 7, WPAD), dtype=np.float32)
        wpack[:BS, :, 0, :BS] = wT(w10h)      # A0: + b1r row + const col
        wpack[BS, :, 0, :BS] = b1[0] * 0.5
        wpack[BS, :, 0, BS] = 1.0
        wpack[:BS, :, 1, :BS] = wT(w11h)      # A1
        wpack[:BS, :, 2, :BS] = wT(-w11h)     # B1
        wpack[:BS, :, 3, :BS] = wT(D1h)       # C1: + b1i row + const col
        wpack[BS, :, 3, :BS] = b1[1] * 0.5
        wpack[BS, :, 3, BS] = 1.0
        wpack[:BS, :, 4, :BS] = wT(A2h)       # Z0: + b2r row
        wpack[BS, :, 4, :BS] = b2[0] * 0.5
        wpack[:BS, :, 5, :BS] = wT(D2h)       # Z1
        wpack[:BS, :, 6, :BS] = wT(D2h * sgn[:, None, None])  # Z1n
        wpack8 = wpack.reshape(P, NB * 7 * WPAD).astype(NPF8)

    bpack = np.empty((P, NB), dtype=np.float32)
    bpack[:BS] = b1[1].T                      # 2*b1i
    bpack[BS] = 0.5
    in_maps = []
    for c in range(NCORES):
        in_maps.append({
            "x": np.ascontiguousarray(sd[c].reshape(NGRP, P, NB * 2 * TILE)),
            "w": wpack8,
            "b": bpack,
        })
    return in_maps


A_, B_, C_, ZT_, ZN_ = range(5)


def _fixed_out(xf, w1, b1, w2, b2, fixed):
    xs = xf[fixed].reshape(len(fixed), NB, BS)
    o1r = np.maximum(np.einsum("knp,npq->knq", xs, w1[0]) + 0.5 * b1[0], 0.0)
    o1i = np.maximum(0.5 * b1[1], 0.0)[None]
    A2h = (w2[0] + w2[1]) * 0.5
    D2h = (w2[0] - w2[1]) * 0.5
    z = (np.einsum("knp,npq->knq", o1r, A2h)
         + np.einsum("knp,npq->knq", np.broadcast_to(o1i, o1r.shape), D2h)
         + 0.5 * b2[0])
    return (z - np.clip(z, -LAM, LAM)).reshape(len(fixed), C)


def _assemble(results, x, w1, b1, w2, b2):
    F, M, fixed = _cache["order"]
    full = np.zeros((SITES, C), dtype=np.float32)
    for c in range(NCORES):
        o = results[c]["out"].reshape(NGRP, BS, NB, 2, TILE)
        t = o[:, :, :, 0, :].transpose(0, 3, 2, 1).reshape(NGRP * TILE, C)
        tn = o[:, :, :, 1, :].transpose(0, 3, 2, 1).reshape(NGRP * TILE, C)
        full[F[c]] = t.astype(np.float32)
        full[M[c]] = tn.astype(np.float32)
    xf = x.reshape(SITES, C)
    full[fixed] = _fixed_out(xf, w1, b1, w2, b2, fixed)
    return (xf + full).reshape(B, N, C)


def _run(x, w1, b1, w2, b2, trace=False):
    if "nc" not in _cache:
        _cache["nc"] = _build()
    nc = _cache["nc"]
    in_maps = _host_prep(x, w1, b1, w2, b2)
    res = bass_utils.run_bass_kernel_spmd(
        nc, in_maps, core_ids=list(range(NCORES)), trace=trace)
    return _assemble(res.results, x, w1, b1, w2, b2), res


def kernel(x, w1, b1, w2, b2):
    out, _ = _run(x, w1, b1, w2, b2, trace=False)
    return out
